# revision 24
# baseline (speedup 1.0000x reference)
"""CapsNet (conv + squash + 3 routed capsule layers + class capsule layer)
on 8 NeuronCores, pure data-parallel over batch (128 -> 8 x 16).

Gram-matrix routing restructure (no pred materialization):
    hc[c]  = sum_i c_coef[i] * h[c,i]
    y      = G_o hc   (G_o = W_o^T W_o, symmetric, host-precomputed)
    n2     = sum_c hc*y ;  u = factor * y ;  db = u^T h
    last round: s = W_o^T hc, v = factor * s
v3 throughput design:
 - every matmul uses a 128x128 block-diagonal stationary in bf16
   (FWL-eligible): h/h^T sample pairs for hc/db, diag(G_o, G_o) /
   diag(W_o^T, W_o^T) for y (both batch-parity halves of one capsule in
   a single N=8 matmul streaming the hc layout directly)
 - ALL round tensors share the batch-parity partition split
   [c + 64*(b%2), ...]; no cross-partition reshuffles inside a round
 - logits live in a PSUM bank; db matmuls accumulate onto them
 - squash factor ~= sqrt(n2+eps)/(1+n2) as two parallel 2-op chains
   (ACT: Ln(+eps)->Exp(0.5x); DVE: (1+n2)->fast reciprocal); exact
   wherever n2 >> eps, and under flush-to-zero decay the output
   stays < 1e-17 (the reference output is exactly 0 there)
 - one activation-table set (natural_log_exp_and_others) -> a single
   ACT_TABLE_LOAD for the whole kernel
 - o-chunked pipeline so PE/Vector/Scalar/GpSimd overlap

Layouts per core (B=16):  b = 2q+p (q<8, p=b%2).
Logits pbl[i + 64p, q*o_n + o].  phc/hcs[c + 64p, q*o_n + o].
py2/z/pn2/fac2[c + 64p, o*8 + q].  u2[c + 64p, q*o_n + o].
h as block-diag pairs: hbd[c + 64p, q*128 + 64p + i] (db lhsT),
htbd[i + 64p, q*128 + 64p + c] (hc lhsT).
b1/b2 are zeros per the problem spec; bb applied in the conv relu.
"""

import sys
import numpy as np

for _p in ("/opt/trn_rl_repo",):
    if _p not in sys.path:
        sys.path.insert(0, _p)

NCORES = 8
B = 16          # batch per core
EPS = 1e-8

_PROG_CACHE = {}


def _build_nc():
    from contextlib import ExitStack
    import concourse.bass as bass
    import concourse.tile as tile
    from concourse import bacc, mybir
    from concourse.masks import make_identity
    from concourse.hw_specs import get_activation_tables

    f32 = mybir.dt.float32
    f32r = mybir.dt.float32r
    bf16 = mybir.dt.bfloat16
    AF = mybir.ActivationFunctionType
    ALU = mybir.AluOpType
    AX = mybir.AxisListType.X

    nc = bacc.Bacc(None, target_bir_lowering=False)

    xp_d = nc.dram_tensor("xp", [64, 1600], f32, kind="ExternalInput")
    wbp_d = nc.dram_tensor("wbp", [64, 576], f32, kind="ExternalInput")
    bbp_d = nc.dram_tensor("bbp", [128, 1], f32, kind="ExternalInput")
    gp2_d = nc.dram_tensor("gp2", [128, 8192], bf16, kind="ExternalInput")
    w1tp_d = nc.dram_tensor("w1tp", [128, 8192], bf16, kind="ExternalInput")
    g2p2_d = nc.dram_tensor("g2p2", [128, 1280], bf16, kind="ExternalInput")
    w2tp_d = nc.dram_tensor("w2tp", [128, 1280], bf16, kind="ExternalInput")
    blogp_d = nc.dram_tensor("blogp", [128, 1536], bf16, kind="ExternalInput")
    blog2p_d = nc.dram_tensor("blog2p", [128, 80], bf16, kind="ExternalInput")
    vout_d = nc.dram_tensor("vout", [64, 160], f32, kind="ExternalOutput")

    with tile.TileContext(nc) as tc, ExitStack() as ctx:
        const = ctx.enter_context(tc.tile_pool(name="const", bufs=1))
        big = ctx.enter_context(tc.tile_pool(name="big", bufs=2))
        work = ctx.enter_context(tc.tile_pool(name="work", bufs=2))
        ps_hc = ctx.enter_context(tc.tile_pool(name="ps_hc", bufs=2, space="PSUM"))
        ps_y = ctx.enter_context(tc.tile_pool(name="ps_y", bufs=2, space="PSUM"))
        ps_n2 = ctx.enter_context(tc.tile_pool(name="ps_n2", bufs=1, space="PSUM"))
        ps_bl = ctx.enter_context(tc.tile_pool(name="ps_bl", bufs=1, space="PSUM"))
        ps_m = ctx.enter_context(tc.tile_pool(name="ps_m", bufs=1, space="PSUM"))

        # ---- one activation-table set that covers Exp+Ln+Square ----
        tabs = list(get_activation_tables(nc.m.arch).keys())
        set_id = tabs.index("natural_log_exp_and_others")
        nc.scalar.add_instruction(mybir.InstLoadActFuncSet(
            name=nc.get_next_instruction_name(), ins=[], outs=[],
            act_func_set_id=set_id))

        # ---- constants / weights ----
        xp = const.tile([64, 1600], f32, tag="xp")
        wbp = const.tile([64, 576], f32, tag="wbp")
        bbp = const.tile([128, 1], f32, tag="bbp")
        gp2 = const.tile([128, 8192], bf16, tag="gp2")
        w1tp = const.tile([128, 8192], bf16, tag="w1tp")
        g2p2 = const.tile([128, 1280], bf16, tag="g2p2")
        w2tp = const.tile([128, 1280], bf16, tag="w2tp")
        blogp = const.tile([128, 1536], bf16, tag="blogp")
        blog2p = const.tile([128, 80], bf16, tag="blog2p")
        nc.sync.dma_start(out=xp, in_=xp_d[:, :])
        nc.sync.dma_start(out=wbp, in_=wbp_d[:, :])
        nc.sync.dma_start(out=bbp, in_=bbp_d[:, :])
        nc.sync.dma_start(out=gp2, in_=gp2_d[:, :])
        nc.sync.dma_start(out=w1tp, in_=w1tp_d[:, :])
        nc.sync.dma_start(out=g2p2, in_=g2p2_d[:, :])
        nc.sync.dma_start(out=w2tp, in_=w2tp_d[:, :])
        nc.sync.dma_start(out=blogp, in_=blogp_d[:, :])
        nc.sync.dma_start(out=blog2p, in_=blog2p_d[:, :])

        # block-diag ones (column-sum within each partition half)
        onesbd = const.tile([128, 128], bf16, tag="onesbd")
        nc.gpsimd.memset(onesbd, 0.0)
        nc.gpsimd.memset(onesbd[0:64, 0:64], 1.0)
        nc.gpsimd.memset(onesbd[64:128, 64:128], 1.0)
        # identities: [128,64] on both halves (transposes), [128,128] (logit init)
        ident2 = const.tile([128, 64], bf16, tag="ident2")
        make_identity(nc, ident2[0:64, :])
        make_identity(nc, ident2[64:128, :])
        ident128 = const.tile([128, 128], bf16, tag="ident128")
        make_identity(nc, ident128)
        for cval in (0.0, EPS):
            cap = const.tile([128, 1], f32, tag=f"c{cval}")
            nc.vector.memset(cap, cval)
            nc.const_aps.aps[(f32, cval)] = cap[:, :]

        actwarm = const.tile([128, 1], f32, tag="actwarm")
        nc.scalar.activation(actwarm, bbp, AF.Exp)

        # fp32r operands for the conv (full-rate fp32 at N>=256)
        xpr = const.tile([64, 1600], f32r, tag="xpr")
        nc.scalar.copy(xpr, xp)
        wbpr = const.tile([64, 576], f32r, tag="wbpr")
        nc.scalar.copy(wbpr, wbp)

        # ---- helpers ----
        def squash_factor(pn2_ap, tiles, sl):
            """fac2 ~= sqrt(n2+eps)/(1+n2) on cols sl.
            Two parallel 2-op chains: ACT Ln(+eps)->Exp(0.5x) and
            DVE (1+n2)->reciprocal_approx_fast; then one multiply."""
            lr, invr, ar, rec, fac2 = tiles
            nc.scalar.activation(lr[:, sl], pn2_ap[:, sl], AF.Ln, bias=EPS)
            nc.scalar.activation(invr[:, sl], lr[:, sl], AF.Exp, scale=0.5)
            nc.scalar.activation(ar[:, sl], pn2_ap[:, sl], AF.Copy, bias=1.0)
            nc.vector.reciprocal_approx_fast(out=rec[:, sl], in_=ar[:, sl])
            nc.gpsimd.tensor_mul(fac2[:, sl], invr[:, sl], rec[:, sl])

        def build_htbd(hbd_t, htbd_t):
            """PE-transpose the 16 h blocks of hbd into htbd. Concurrent
            matmuls with different row groups may not share a PSUM bank,
            so one staging bank per parity; then 2 copies (DVE)."""
            pta = ps_m.tile([64, 1024], bf16, tag="pt")
            ptb = ps_m.tile([64, 1024], bf16, tag="ptb")
            pts = (pta, ptb)
            for p in range(2):
                for q in range(8):
                    in_ap = hbd_t[64 * p:64 * p + 64,
                                  q * 128 + 64 * p:q * 128 + 64 * p + 64]
                    nc.tensor.transpose(
                        pts[p][:, q * 64:q * 64 + 64],
                        in_ap, ident2[64 * p:64 * p + 64, :])
            for p in range(2):
                # htbd[i+64p, q*128 + 64p + c] <- pts[p][i, q*64 + c]
                out_ap = htbd_t[64 * p:64 * p + 64].rearrange(
                    "i (q pp c) -> i q pp c", q=8, pp=2)[:, :, p]
                in_ap = pts[p][:, 0:512].rearrange(
                    "i (q c) -> i q c", q=8)
                nc.vector.tensor_copy(out=out_ap, in_=in_ap)

        # ---- conv 3x3 SAME (64->64 ch over 8x8) + relu(+bb) + ch-squash ----
        # f32r replication is incompatible with col tile_position: the two
        # batch halves use separate 64-partition PSUM tiles, merged at relu.
        pca = ps_hc.tile([64, 512], f32, tag="phc")
        pcb = ps_y.tile([64, 512], f32, tag="py")
        xv = xpr.rearrange("p (b h w) -> p b h w", b=16, h=10, w=10)
        for g, pcg in ((0, pca), (1, pcb)):
            cvg = pcg.rearrange("p (b h w) -> p b h w", b=8, h=8, w=8)
            for it in range(9):
                ky, kx = it // 3, it % 3
                nc.tensor.matmul(
                    out=cvg,
                    lhsT=wbpr[:, it * 64:(it + 1) * 64],
                    rhs=xv[:, g * 8:(g + 1) * 8, ky:ky + 8, kx:kx + 8],
                    start=(it == 0), stop=(it == 8),
                )
        h_raw = work.tile([128, 512], f32, tag="hraw")
        nc.vector.tensor_scalar(out=h_raw[0:64, :], in0=pca,
                                scalar1=bbp[0:64, 0:1], scalar2=0.0,
                                op0=ALU.add, op1=ALU.max)
        nc.scalar.activation(h_raw[64:128, :], pcb, AF.Relu,
                             bias=bbp[64:128, 0:1])
        h2 = work.tile([128, 512], bf16, tag="h2")
        nc.gpsimd.tensor_mul(h2, h_raw, h_raw)
        pn2c = ps_n2.tile([128, 512], f32, tag="pn2")
        nc.tensor.matmul(out=pn2c, lhsT=onesbd, rhs=h2)
        # conv squash uses the exact factor n2*((n2+eps)(1+n2)^2)^-0.5
        arC = work.tile([128, 512], f32, tag="ar")
        nc.scalar.activation(arC, pn2c, AF.Copy, bias=1.0)
        stC = work.tile([128, 512], f32, tag="st")
        nc.vector.scalar_tensor_tensor(out=stC, in0=pn2c, scalar=EPS,
                                       in1=arC, op0=ALU.add, op1=ALU.mult)
        urC = work.tile([128, 512], f32, tag="ur")
        nc.gpsimd.tensor_mul(urC, stC, arC)
        lrC = work.tile([128, 512], f32, tag="lr")
        nc.scalar.activation(lrC, urC, AF.Ln)
        ivC = work.tile([128, 512], bf16, tag="iv")
        nc.scalar.activation(ivC, lrC, AF.Exp, scale=-0.5)
        facc = work.tile([128, 512], bf16, tag="fc")
        nc.vector.tensor_mul(facc, pn2c, ivC)

        hbd0 = big.tile([128, 1024], bf16, tag="hbd")
        htbd0 = big.tile([128, 1024], bf16, tag="htbd")
        nc.gpsimd.memset(hbd0, 0.0)
        nc.vector.memset(htbd0, 0.0)
        engs = [nc.vector, nc.gpsimd, nc.vector, nc.gpsimd]
        k = 0
        for g in range(2):
            for p in range(2):
                # b = 8g + 2(q-4g) + p ; out col q*128 + 64p + i
                out_ap = hbd0[64 * p:64 * p + 64].rearrange(
                    "c (q pp i) -> c q pp i", q=8, pp=2)[:, 4 * g:4 * g + 4, p]
                in_h = h_raw[64 * g:64 * g + 64].rearrange(
                    "c (qq pp i) -> c qq pp i", qq=4, pp=2)[:, :, p]
                in_f = facc[64 * g:64 * g + 64].rearrange(
                    "c (qq pp i) -> c qq pp i", qq=4, pp=2)[:, :, p]
                engs[k].tensor_tensor(out=out_ap, in0=in_h, in1=in_f,
                                      op=ALU.mult)
                k += 1
        build_htbd(hbd0, htbd0)

        # ---- one routed capsule layer (3 routing rounds) ----
        def routing_layer(lname, o_n, bl0_ap, gpair, wtpair, hbd_in, htbd_in,
                          hbd_out, htbd_out, vout_sb):
            w = 8 * o_n
            nch = 2 if o_n >= 16 else 1
            och = o_n // nch
            # logits into a PSUM bank; db matmuls accumulate onto it
            pbl = ps_bl.tile([128, w], f32, tag="pbl")
            nc.tensor.matmul(out=pbl, lhsT=ident128, rhs=bl0_ap,
                             skip_group_check=True)
            for r3 in range(3):
                last = (r3 == 2)
                # softmax over o, chunked by q-halves so it overlaps db
                e = work.tile([128, w], bf16, tag="e")
                ssum = work.tile([128, 8], f32, tag="ssum")
                rs = work.tile([128, 8], f32, tag="rs")
                cc = work.tile([128, w], bf16, tag="cc")
                for qc in range(2):
                    qs = slice(qc * w // 2, (qc + 1) * w // 2)
                    nc.scalar.activation(e[:, qs], pbl[:, qs], AF.Exp)
                    nc.vector.tensor_reduce(
                        out=ssum[:, qc * 4:qc * 4 + 4],
                        in_=e[:, qs].rearrange("p (q o) -> p q o", o=o_n),
                        axis=AX, op=ALU.add)
                    nc.vector.reciprocal_approx_fast(
                        out=rs[:, qc * 4:qc * 4 + 4],
                        in_=ssum[:, qc * 4:qc * 4 + 4])
                    nc.gpsimd.tensor_tensor(
                        out=cc[:, qs].rearrange("p (q o) -> p q o", o=o_n),
                        in0=e[:, qs].rearrange("p (q o) -> p q o", o=o_n),
                        in1=rs[:, qc * 4:qc * 4 + 4].unsqueeze(2)
                            .broadcast_to([128, 4, o_n]),
                        op=ALU.mult)

                # hc: 8 block-diag pair matmuls, out [c+64p, (q, o)]
                phc = ps_hc.tile([128, w], f32, tag="phc")
                for q in range(8):
                    nc.tensor.matmul(
                        out=phc[:, q * o_n:(q + 1) * o_n],
                        lhsT=htbd_in[:, q * 128:(q + 1) * 128],
                        rhs=cc[:, q * o_n:(q + 1) * o_n])
                # SBUF bf16 copy (y rhs must be SBUF); o-strided chunks so
                # the first y matmuls can start after chunk 0
                hcs = work.tile([128, w], bf16, tag="hcs")
                nc.vector.tensor_copy(out=hcs, in_=phc)
                hcv = hcs.rearrange("c (q o) -> c o q", o=o_n)
                # y (s in last round): one diag(M_o, M_o) matmul per
                # capsule, writing the (q, o) layout via strided out cols
                mat = wtpair if last else gpair
                py2 = ps_y.tile([128, w], f32, tag="py")
                pyv = py2.rearrange("c (q o) -> c o q", o=o_n)
                for o in range(o_n):
                    nc.tensor.matmul(
                        out=pyv[:, o],
                        lhsT=mat[:, o * 128:(o + 1) * 128],
                        rhs=hcv[:, o])
                # whole-width tail: z, n2, squash factor, u
                z = work.tile([128, w], bf16, tag="z")
                if last:
                    nc.scalar.activation(z, py2, AF.Square)
                else:
                    nc.vector.tensor_tensor(out=z, in0=py2, in1=hcs,
                                            op=ALU.mult)
                pn2 = ps_n2.tile([128, w], f32, tag="pn2")
                nc.tensor.matmul(out=pn2, lhsT=onesbd, rhs=z)
                lr = work.tile([128, w], f32, tag="lr")
                invr = work.tile([128, w], bf16, tag="iv")
                ar = work.tile([128, w], f32, tag="ar")
                rec = work.tile([128, w], f32, tag="rec")
                fac2 = work.tile([128, w], bf16, tag="fc")
                squash_factor(pn2, (lr, invr, ar, rec, fac2), slice(0, w))
                if not last:
                    u2 = work.tile([128, w], bf16, tag="u2")
                    for qc in range(2):
                        qs = slice(qc * w // 2, (qc + 1) * w // 2)
                        nc.vector.tensor_mul(u2[:, qs], py2[:, qs],
                                             fac2[:, qs])
                        # db accumulates onto the logit bank
                        for q in range(qc * 4, qc * 4 + 4):
                            nc.tensor.matmul(
                                out=pbl[:, q * o_n:(q + 1) * o_n],
                                lhsT=hbd_in[:, q * 128:(q + 1) * 128],
                                rhs=u2[:, q * o_n:(q + 1) * o_n],
                                start=False, stop=(r3 == 1),
                                skip_group_check=True)
                elif vout_sb is not None:
                    # final class layer: v = factor*s -> [d, b*10 + o]
                    for p in range(2):
                        out_ap = vout_sb.rearrange(
                            "d (q pp o) -> d pp q o", q=8, pp=2)[:, p]
                        in0 = py2[64 * p:64 * p + 64].rearrange(
                            "d (q o) -> d q o", o=o_n)
                        in1 = fac2[64 * p:64 * p + 64].rearrange(
                            "d (q o) -> d q o", o=o_n)
                        nc.vector.tensor_tensor(out=out_ap, in0=in0, in1=in1,
                                                op=ALU.mult)
                else:
                    # v = factor*s into next layer's block-diag h
                    for p in range(2):
                        out_ap = hbd_out[64 * p:64 * p + 64].rearrange(
                            "c (q pp i) -> c pp q i", q=8, pp=2)[:, p]
                        in0 = py2[64 * p:64 * p + 64].rearrange(
                            "c (q o) -> c q o", o=o_n)
                        in1 = fac2[64 * p:64 * p + 64].rearrange(
                            "c (q o) -> c q o", o=o_n)
                        nc.vector.tensor_tensor(out=out_ap, in0=in0,
                                                in1=in1, op=ALU.mult)
                    build_htbd(hbd_out, htbd_out)

        # ---- 3 basic layers + final class layer ----
        hbd_c, htbd_c = hbd0, htbd0
        for l in range(3):
            hbd_n = big.tile([128, 1024], bf16, tag="hbd")
            htbd_n = big.tile([128, 1024], bf16, tag="htbd")
            if l == 0:
                # bufs=2: first two allocations of each tag need zeroing;
                # later ones inherit the (never-overwritten) zero blocks
                nc.gpsimd.memset(hbd_n, 0.0)
                nc.vector.memset(htbd_n, 0.0)
            routing_layer(f"L{l}", 64, blogp[:, l * 512:(l + 1) * 512],
                          gp2, w1tp, hbd_c, htbd_c, hbd_n, htbd_n, None)
            hbd_c, htbd_c = hbd_n, htbd_n
        vout_sb = work.tile([64, 160], f32, tag="vo")
        routing_layer("F", 10, blog2p, g2p2, w2tp, hbd_c, htbd_c,
                      None, None, vout_sb)
        nc.sync.dma_start(out=vout_d[:, :], in_=vout_sb)

    nc.compile()
    return nc


def _prep_inputs(x, Wb, bb, W1, W2, b_basic, b_cls):
    """Host-side shard + relayout. Returns list of per-core input dicts."""
    import ml_dtypes
    f = np.float32
    bf = ml_dtypes.bfloat16

    wbp = np.ascontiguousarray(Wb.transpose(1, 2, 3, 0).reshape(64, 576), f)
    bbp = np.ascontiguousarray(np.tile(bb.reshape(1, 64), (2, 1))
                               .reshape(128, 1), f)

    def dup_blockdiag(mats):
        # mats: [o, 64, 64] -> [128, o*128] with diag(M_o, M_o)
        n = mats.shape[0]
        out = np.zeros((n, 128, 128), f)
        out[:, :64, :64] = mats
        out[:, 64:, 64:] = mats
        return np.ascontiguousarray(
            out.transpose(1, 0, 2).reshape(128, n * 128)).astype(bf)

    w1r = W1.reshape(64, 64, 64)                          # [o, d, c]
    g1 = np.einsum("odc,ode->oce", w1r, w1r)              # [o, c, e]
    gp2 = dup_blockdiag(g1)
    w1tp = dup_blockdiag(w1r.transpose(0, 2, 1))          # lhsT = [c, d]
    w2r = W2.reshape(10, 64, 64)
    g2 = np.einsum("odc,ode->oce", w2r, w2r)
    g2p2 = dup_blockdiag(g2)
    w2tp = dup_blockdiag(w2r.transpose(0, 2, 1))

    maps = []
    for core in range(NCORES):
        s = slice(core * B, (core + 1) * B)
        xs = x[s]                                         # [16,64,8,8]
        xpad = np.zeros((64, B, 10, 10), f)
        xpad[:, :, 1:9, 1:9] = xs.transpose(1, 0, 2, 3)
        xp = np.ascontiguousarray(xpad.reshape(64, 1600), f)
        # blogp[i + 64p, l*512 + q*64 + o] = b_basic[l, 2q+p, o, i]
        bs = b_basic[:, s]                                # [3,16,64,64]
        bq = bs.reshape(3, 8, 2, 64, 64)                  # [l,q,p,o,i]
        blogp = np.ascontiguousarray(
            bq.transpose(2, 4, 0, 1, 3).reshape(128, 1536)).astype(bf)
        cs = b_cls[s].reshape(8, 2, 10, 64)               # [q,p,o,i]
        blog2p = np.ascontiguousarray(
            cs.transpose(1, 3, 0, 2).reshape(128, 80)).astype(bf)
        maps.append(dict(xp=xp, wbp=wbp, bbp=bbp, gp2=gp2, w1tp=w1tp,
                         g2p2=g2p2, w2tp=w2tp, blogp=blogp, blog2p=blog2p))
    return maps


def kernel(x, Wb, bb, W1, b1, W2, b2, b_basic, b_cls):
    from concourse.bass_utils import run_bass_kernel_spmd

    if "nc" not in _PROG_CACHE:
        _PROG_CACHE["nc"] = _build_nc()
    nc = _PROG_CACHE["nc"]

    in_maps = _prep_inputs(np.asarray(x), np.asarray(Wb), np.asarray(bb),
                           np.asarray(W1), np.asarray(W2),
                           np.asarray(b_basic), np.asarray(b_cls))
    res = run_bass_kernel_spmd(nc, in_maps, list(range(NCORES)))
    out = np.empty((128, 10, 64), np.float32)
    for core in range(NCORES):
        vo = res.results[core]["vout"]                    # [64, 160]
        out[core * B:(core + 1) * B] = vo.reshape(64, B, 10).transpose(1, 2, 0)
    return out


# revision 25
# speedup vs baseline: 1.0706x; 1.0706x over previous
"""CapsNet (conv + squash + 3 routed capsule layers + class capsule layer)
on 8 NeuronCores, pure data-parallel over batch (128 -> 8 x 16).

Gram-matrix routing restructure (no pred materialization):
    hc[c]  = sum_i c_coef[i] * h[c,i]
    y      = G_o hc   (G_o = W_o^T W_o, symmetric, host-precomputed)
    n2     = sum_c hc*y ;  u = factor * y ;  db = u^T h
    last round: s = W_o^T hc, v = factor * s
v3 throughput design:
 - every matmul uses a 128x128 block-diagonal stationary in bf16
   (FWL-eligible): h/h^T sample pairs for hc/db, diag(G_o, G_o) /
   diag(W_o^T, W_o^T) for y (both batch-parity halves of one capsule in
   a single N=8 matmul streaming the hc layout directly)
 - ALL round tensors share the batch-parity partition split
   [c + 64*(b%2), ...]; no cross-partition reshuffles inside a round
 - logits live in a PSUM bank; db matmuls accumulate onto them
 - squash factor ~= sqrt(n2+eps)/(1+n2) as two parallel 2-op chains
   (ACT: Ln(+eps)->Exp(0.5x); DVE: (1+n2)->fast reciprocal); exact
   wherever n2 >> eps, and under flush-to-zero decay the output
   stays < 1e-17 (the reference output is exactly 0 there)
 - one activation-table set (natural_log_exp_and_others) -> a single
   ACT_TABLE_LOAD for the whole kernel
 - o-chunked pipeline so PE/Vector/Scalar/GpSimd overlap

Layouts per core (B=16):  b = 2q+p (q<8, p=b%2).
Logits pbl[i + 64p, q*o_n + o].  phc/hcs[c + 64p, q*o_n + o].
py2/z/pn2/fac2[c + 64p, o*8 + q].  u2[c + 64p, q*o_n + o].
h as block-diag pairs: hbd[c + 64p, q*128 + 64p + i] (db lhsT),
htbd[i + 64p, q*128 + 64p + c] (hc lhsT).
b1/b2 are zeros per the problem spec; bb applied in the conv relu.
"""

import sys
import numpy as np

for _p in ("/opt/trn_rl_repo",):
    if _p not in sys.path:
        sys.path.insert(0, _p)

NCORES = 8
B = 16          # batch per core
EPS = 1e-8

_PROG_CACHE = {}


def _build_nc():
    from contextlib import ExitStack
    import concourse.bass as bass
    import concourse.tile as tile
    from concourse import bacc, mybir
    from concourse.masks import make_identity
    from concourse.hw_specs import get_activation_tables

    f32 = mybir.dt.float32
    f32r = mybir.dt.float32r
    bf16 = mybir.dt.bfloat16
    AF = mybir.ActivationFunctionType
    ALU = mybir.AluOpType
    AX = mybir.AxisListType.X

    nc = bacc.Bacc(None, target_bir_lowering=False)

    xp_d = nc.dram_tensor("xp", [64, 1600], f32, kind="ExternalInput")
    wbp_d = nc.dram_tensor("wbp", [64, 576], f32, kind="ExternalInput")
    bbp_d = nc.dram_tensor("bbp", [128, 1], f32, kind="ExternalInput")
    gp2_d = nc.dram_tensor("gp2", [128, 8192], bf16, kind="ExternalInput")
    w1tp_d = nc.dram_tensor("w1tp", [128, 8192], bf16, kind="ExternalInput")
    g2p2_d = nc.dram_tensor("g2p2", [128, 1280], bf16, kind="ExternalInput")
    w2tp_d = nc.dram_tensor("w2tp", [128, 1280], bf16, kind="ExternalInput")
    blogp_d = nc.dram_tensor("blogp", [128, 1536], bf16, kind="ExternalInput")
    blog2p_d = nc.dram_tensor("blog2p", [128, 80], bf16, kind="ExternalInput")
    vout_d = nc.dram_tensor("vout", [64, 160], f32, kind="ExternalOutput")

    with tile.TileContext(nc) as tc, ExitStack() as ctx:
        const = ctx.enter_context(tc.tile_pool(name="const", bufs=1))
        big = ctx.enter_context(tc.tile_pool(name="big", bufs=2))
        work = ctx.enter_context(tc.tile_pool(name="work", bufs=2))
        ps_hc = ctx.enter_context(tc.tile_pool(name="ps_hc", bufs=2, space="PSUM"))
        ps_y = ctx.enter_context(tc.tile_pool(name="ps_y", bufs=2, space="PSUM"))
        ps_n2 = ctx.enter_context(tc.tile_pool(name="ps_n2", bufs=1, space="PSUM"))
        ps_bl = ctx.enter_context(tc.tile_pool(name="ps_bl", bufs=1, space="PSUM"))
        ps_m = ctx.enter_context(tc.tile_pool(name="ps_m", bufs=1, space="PSUM"))

        # ---- one activation-table set that covers Exp+Ln+Square ----
        tabs = list(get_activation_tables(nc.m.arch).keys())
        set_id = tabs.index("natural_log_exp_and_others")
        nc.scalar.add_instruction(mybir.InstLoadActFuncSet(
            name=nc.get_next_instruction_name(), ins=[], outs=[],
            act_func_set_id=set_id))

        # ---- constants / weights ----
        xp = const.tile([64, 1600], f32, tag="xp")
        wbp = const.tile([64, 576], f32, tag="wbp")
        bbp = const.tile([128, 1], f32, tag="bbp")
        gp2 = const.tile([128, 8192], bf16, tag="gp2")
        w1tp = const.tile([128, 8192], bf16, tag="w1tp")
        g2p2 = const.tile([128, 1280], bf16, tag="g2p2")
        w2tp = const.tile([128, 1280], bf16, tag="w2tp")
        blogp = const.tile([128, 1536], bf16, tag="blogp")
        blog2p = const.tile([128, 80], bf16, tag="blog2p")
        nc.sync.dma_start(out=xp, in_=xp_d[:, :])
        nc.sync.dma_start(out=wbp, in_=wbp_d[:, :])
        nc.sync.dma_start(out=bbp, in_=bbp_d[:, :])
        nc.sync.dma_start(out=gp2, in_=gp2_d[:, :])
        nc.sync.dma_start(out=w1tp, in_=w1tp_d[:, :])
        nc.sync.dma_start(out=g2p2, in_=g2p2_d[:, :])
        nc.sync.dma_start(out=w2tp, in_=w2tp_d[:, :])
        nc.sync.dma_start(out=blogp, in_=blogp_d[:, :])
        nc.sync.dma_start(out=blog2p, in_=blog2p_d[:, :])

        # block-diag ones (column-sum within each partition half)
        onesbd = const.tile([128, 128], bf16, tag="onesbd")
        nc.gpsimd.memset(onesbd, 0.0)
        nc.gpsimd.memset(onesbd[0:64, 0:64], 1.0)
        nc.gpsimd.memset(onesbd[64:128, 64:128], 1.0)
        # identities: [128,64] on both halves (transposes), [128,128] (logit init)
        ident2 = const.tile([128, 64], bf16, tag="ident2")
        make_identity(nc, ident2[0:64, :])
        make_identity(nc, ident2[64:128, :])
        ident128 = const.tile([128, 128], bf16, tag="ident128")
        make_identity(nc, ident128)
        for cval in (0.0, EPS):
            cap = const.tile([128, 1], f32, tag=f"c{cval}")
            nc.vector.memset(cap, cval)
            nc.const_aps.aps[(f32, cval)] = cap[:, :]

        actwarm = const.tile([128, 1], f32, tag="actwarm")
        nc.scalar.activation(actwarm, bbp, AF.Exp)

        # fp32r operands for the conv (full-rate fp32 at N>=256)
        xpr = const.tile([64, 1600], f32r, tag="xpr")
        nc.scalar.copy(xpr, xp)
        wbpr = const.tile([64, 576], f32r, tag="wbpr")
        nc.scalar.copy(wbpr, wbp)

        # ---- helpers ----
        def squash_factor(pn2_ap, tiles, sl):
            """fac2 ~= sqrt(n2+eps)/(1+n2) on cols sl.
            Two parallel 2-op chains: ACT Ln(+eps)->Exp(0.5x) and
            DVE (1+n2)->reciprocal_approx_fast; then one multiply."""
            lr, invr, ar, rec, fac2 = tiles
            nc.scalar.activation(lr[:, sl], pn2_ap[:, sl], AF.Ln, bias=EPS)
            nc.scalar.activation(invr[:, sl], lr[:, sl], AF.Exp, scale=0.5)
            nc.vector.tensor_scalar_add(ar[:, sl], pn2_ap[:, sl], 1.0)
            nc.vector.reciprocal_approx_fast(out=rec[:, sl], in_=ar[:, sl])
            nc.vector.tensor_mul(fac2[:, sl], invr[:, sl], rec[:, sl])

        def build_htbd(hbd_t, htbd_t):
            """PE-transpose the 16 h blocks of hbd into htbd. Concurrent
            matmuls with different row groups may not share a PSUM bank,
            so one staging bank per parity; then 2 copies (DVE)."""
            pta = ps_m.tile([64, 1024], bf16, tag="pt")
            ptb = ps_m.tile([64, 1024], bf16, tag="ptb")
            pts = (pta, ptb)
            for p in range(2):
                for q in range(8):
                    in_ap = hbd_t[64 * p:64 * p + 64,
                                  q * 128 + 64 * p:q * 128 + 64 * p + 64]
                    nc.tensor.transpose(
                        pts[p][:, q * 64:q * 64 + 64],
                        in_ap, ident2[64 * p:64 * p + 64, :])
            for p in range(2):
                # htbd[i+64p, q*128 + 64p + c] <- pts[p][i, q*64 + c]
                out_ap = htbd_t[64 * p:64 * p + 64].rearrange(
                    "i (q pp c) -> i q pp c", q=8, pp=2)[:, :, p]
                in_ap = pts[p][:, 0:512].rearrange(
                    "i (q c) -> i q c", q=8)
                nc.vector.tensor_copy(out=out_ap, in_=in_ap)

        # ---- conv 3x3 SAME (64->64 ch over 8x8) + relu(+bb) + ch-squash ----
        # f32r replication is incompatible with col tile_position: the two
        # batch halves use separate 64-partition PSUM tiles, merged at relu.
        pca = ps_hc.tile([64, 512], f32, tag="phc")
        pcb = ps_y.tile([64, 512], f32, tag="py")
        xv = xpr.rearrange("p (b h w) -> p b h w", b=16, h=10, w=10)
        for g, pcg in ((0, pca), (1, pcb)):
            cvg = pcg.rearrange("p (b h w) -> p b h w", b=8, h=8, w=8)
            for it in range(9):
                ky, kx = it // 3, it % 3
                nc.tensor.matmul(
                    out=cvg,
                    lhsT=wbpr[:, it * 64:(it + 1) * 64],
                    rhs=xv[:, g * 8:(g + 1) * 8, ky:ky + 8, kx:kx + 8],
                    start=(it == 0), stop=(it == 8),
                )
        h_raw = work.tile([128, 512], f32, tag="hraw")
        nc.vector.tensor_scalar(out=h_raw[0:64, :], in0=pca,
                                scalar1=bbp[0:64, 0:1], scalar2=0.0,
                                op0=ALU.add, op1=ALU.max)
        nc.scalar.activation(h_raw[64:128, :], pcb, AF.Relu,
                             bias=bbp[64:128, 0:1])
        h2 = work.tile([128, 512], bf16, tag="h2")
        nc.gpsimd.tensor_mul(h2, h_raw, h_raw)
        pn2c = ps_n2.tile([128, 512], f32, tag="pn2")
        nc.tensor.matmul(out=pn2c, lhsT=onesbd, rhs=h2)
        # conv squash uses the exact factor n2*((n2+eps)(1+n2)^2)^-0.5
        arC = work.tile([128, 512], f32, tag="ar")
        nc.scalar.activation(arC, pn2c, AF.Copy, bias=1.0)
        stC = work.tile([128, 512], f32, tag="st")
        nc.vector.scalar_tensor_tensor(out=stC, in0=pn2c, scalar=EPS,
                                       in1=arC, op0=ALU.add, op1=ALU.mult)
        urC = work.tile([128, 512], f32, tag="ur")
        nc.gpsimd.tensor_mul(urC, stC, arC)
        lrC = work.tile([128, 512], f32, tag="lr")
        nc.scalar.activation(lrC, urC, AF.Ln)
        ivC = work.tile([128, 512], bf16, tag="iv")
        nc.scalar.activation(ivC, lrC, AF.Exp, scale=-0.5)
        facc = work.tile([128, 512], bf16, tag="fc")
        nc.vector.tensor_mul(facc, pn2c, ivC)

        hbd0 = big.tile([128, 1024], bf16, tag="hbd")
        htbd0 = big.tile([128, 1024], bf16, tag="htbd")
        nc.gpsimd.memset(hbd0, 0.0)
        nc.vector.memset(htbd0, 0.0)
        engs = [nc.vector, nc.gpsimd, nc.vector, nc.gpsimd]
        k = 0
        for g in range(2):
            for p in range(2):
                # b = 8g + 2(q-4g) + p ; out col q*128 + 64p + i
                out_ap = hbd0[64 * p:64 * p + 64].rearrange(
                    "c (q pp i) -> c q pp i", q=8, pp=2)[:, 4 * g:4 * g + 4, p]
                in_h = h_raw[64 * g:64 * g + 64].rearrange(
                    "c (qq pp i) -> c qq pp i", qq=4, pp=2)[:, :, p]
                in_f = facc[64 * g:64 * g + 64].rearrange(
                    "c (qq pp i) -> c qq pp i", qq=4, pp=2)[:, :, p]
                engs[k].tensor_tensor(out=out_ap, in0=in_h, in1=in_f,
                                      op=ALU.mult)
                k += 1
        build_htbd(hbd0, htbd0)

        # ---- one routed capsule layer (3 routing rounds) ----
        def routing_layer(lname, o_n, bl0_ap, gpair, wtpair, hbd_in, htbd_in,
                          hbd_out, htbd_out, vout_sb):
            w = 8 * o_n
            nch = 2 if o_n >= 16 else 1
            och = o_n // nch
            # logits into a PSUM bank; db matmuls accumulate onto it
            pbl = ps_bl.tile([128, w], f32, tag="pbl")
            nc.tensor.matmul(out=pbl, lhsT=ident128, rhs=bl0_ap,
                             skip_group_check=True)
            for r3 in range(3):
                last = (r3 == 2)
                # softmax over o, chunked by q-halves so it overlaps db
                e = work.tile([128, w], bf16, tag="e")
                ssum = work.tile([128, 8], f32, tag="ssum")
                rs = work.tile([128, 8], f32, tag="rs")
                cc = work.tile([128, w], bf16, tag="cc")
                for qc in range(2):
                    qs = slice(qc * w // 2, (qc + 1) * w // 2)
                    nc.scalar.activation(e[:, qs], pbl[:, qs], AF.Exp)
                    nc.vector.tensor_reduce(
                        out=ssum[:, qc * 4:qc * 4 + 4],
                        in_=e[:, qs].rearrange("p (q o) -> p q o", o=o_n),
                        axis=AX, op=ALU.add)
                    nc.vector.reciprocal_approx_fast(
                        out=rs[:, qc * 4:qc * 4 + 4],
                        in_=ssum[:, qc * 4:qc * 4 + 4])
                    nc.gpsimd.tensor_tensor(
                        out=cc[:, qs].rearrange("p (q o) -> p q o", o=o_n),
                        in0=e[:, qs].rearrange("p (q o) -> p q o", o=o_n),
                        in1=rs[:, qc * 4:qc * 4 + 4].unsqueeze(2)
                            .broadcast_to([128, 4, o_n]),
                        op=ALU.mult)

                # hc: 8 block-diag pair matmuls, out [c+64p, (q, o)]
                phc = ps_hc.tile([128, w], f32, tag="phc")
                for q in range(8):
                    nc.tensor.matmul(
                        out=phc[:, q * o_n:(q + 1) * o_n],
                        lhsT=htbd_in[:, q * 128:(q + 1) * 128],
                        rhs=cc[:, q * o_n:(q + 1) * o_n])
                # SBUF bf16 copy (y rhs must be SBUF); o-strided chunks so
                # the first y matmuls can start after chunk 0
                hcs = work.tile([128, w], bf16, tag="hcs")
                nc.vector.tensor_copy(out=hcs, in_=phc)
                hcv = hcs.rearrange("c (q o) -> c o q", o=o_n)
                # y (s in last round): one diag(M_o, M_o) matmul per
                # capsule, writing the (q, o) layout via strided out cols
                mat = wtpair if last else gpair
                py2 = ps_y.tile([128, w], f32, tag="py")
                pyv = py2.rearrange("c (q o) -> c o q", o=o_n)
                for o in range(o_n):
                    nc.tensor.matmul(
                        out=pyv[:, o],
                        lhsT=mat[:, o * 128:(o + 1) * 128],
                        rhs=hcv[:, o])
                # tail chunked by o-halves: z, n2, squash factor
                z = work.tile([128, w], bf16, tag="z")
                pn2 = ps_n2.tile([128, w], f32, tag="pn2")
                lr = work.tile([128, w], f32, tag="lr")
                invr = work.tile([128, w], bf16, tag="iv")
                ar = work.tile([128, w], f32, tag="ar")
                rec = work.tile([128, w], f32, tag="rec")
                fac2 = work.tile([128, w], bf16, tag="fc")
                zv = z.rearrange("c (q o) -> c q o", o=o_n)
                pnv = pn2.rearrange("c (q o) -> c q o", o=o_n)
                for chn in range(nch):
                    o0, o1 = chn * och, (chn + 1) * och
                    if last:
                        nc.scalar.activation(
                            zv[:, :, o0:o1],
                            py2.rearrange("c (q o) -> c q o", o=o_n)[
                                :, :, o0:o1], AF.Square)
                    else:
                        nc.vector.tensor_tensor(
                            out=zv[:, :, o0:o1],
                            in0=py2.rearrange("c (q o) -> c q o", o=o_n)[
                                :, :, o0:o1],
                            in1=hcs.rearrange("c (q o) -> c q o", o=o_n)[
                                :, :, o0:o1],
                            op=ALU.mult)
                    nc.tensor.matmul(out=pnv[:, :, o0:o1], lhsT=onesbd,
                                     rhs=zv[:, :, o0:o1])
                    sqsl = [slice(0, w), (slice(None), slice(o0, o1))][0] \
                        if nch == 1 else None
                    if nch == 1:
                        squash_factor(pn2, (lr, invr, ar, rec, fac2),
                                      slice(0, w))
                    else:
                        # strided chunk views for the factor chain
                        csl = (slice(None), slice(o0, o1))
                        nc.scalar.activation(
                            lr.rearrange("c (q o) -> c q o", o=o_n)[
                                :, :, o0:o1],
                            pnv[:, :, o0:o1], AF.Ln, bias=EPS)
                        nc.scalar.activation(
                            invr.rearrange("c (q o) -> c q o", o=o_n)[
                                :, :, o0:o1],
                            lr.rearrange("c (q o) -> c q o", o=o_n)[
                                :, :, o0:o1], AF.Exp, scale=0.5)
                        nc.vector.tensor_scalar_add(
                            ar.rearrange("c (q o) -> c q o", o=o_n)[
                                :, :, o0:o1],
                            pnv[:, :, o0:o1], 1.0)
                        nc.vector.reciprocal_approx_fast(
                            out=rec.rearrange("c (q o) -> c q o", o=o_n)[
                                :, :, o0:o1],
                            in_=ar.rearrange("c (q o) -> c q o", o=o_n)[
                                :, :, o0:o1])
                        nc.vector.tensor_mul(
                            fac2.rearrange("c (q o) -> c q o", o=o_n)[
                                :, :, o0:o1],
                            invr.rearrange("c (q o) -> c q o", o=o_n)[
                                :, :, o0:o1],
                            rec.rearrange("c (q o) -> c q o", o=o_n)[
                                :, :, o0:o1])
                if not last:
                    u2 = work.tile([128, w], bf16, tag="u2")
                    for qc in range(2):
                        qs = slice(qc * w // 2, (qc + 1) * w // 2)
                        nc.vector.tensor_mul(u2[:, qs], py2[:, qs],
                                             fac2[:, qs])
                        # db accumulates onto the logit bank
                        for q in range(qc * 4, qc * 4 + 4):
                            nc.tensor.matmul(
                                out=pbl[:, q * o_n:(q + 1) * o_n],
                                lhsT=hbd_in[:, q * 128:(q + 1) * 128],
                                rhs=u2[:, q * o_n:(q + 1) * o_n],
                                start=False, stop=(r3 == 1),
                                skip_group_check=True)
                elif vout_sb is not None:
                    # final class layer: v = factor*s -> [d, b*10 + o]
                    for p in range(2):
                        out_ap = vout_sb.rearrange(
                            "d (q pp o) -> d pp q o", q=8, pp=2)[:, p]
                        in0 = py2[64 * p:64 * p + 64].rearrange(
                            "d (q o) -> d q o", o=o_n)
                        in1 = fac2[64 * p:64 * p + 64].rearrange(
                            "d (q o) -> d q o", o=o_n)
                        nc.vector.tensor_tensor(out=out_ap, in0=in0, in1=in1,
                                                op=ALU.mult)
                else:
                    # v = factor*s into next layer's block-diag h
                    for p in range(2):
                        out_ap = hbd_out[64 * p:64 * p + 64].rearrange(
                            "c (q pp i) -> c pp q i", q=8, pp=2)[:, p]
                        in0 = py2[64 * p:64 * p + 64].rearrange(
                            "c (q o) -> c q o", o=o_n)
                        in1 = fac2[64 * p:64 * p + 64].rearrange(
                            "c (q o) -> c q o", o=o_n)
                        nc.vector.tensor_tensor(out=out_ap, in0=in0,
                                                in1=in1, op=ALU.mult)
                    build_htbd(hbd_out, htbd_out)

        # ---- 3 basic layers + final class layer ----
        hbd_c, htbd_c = hbd0, htbd0
        for l in range(3):
            hbd_n = big.tile([128, 1024], bf16, tag="hbd")
            htbd_n = big.tile([128, 1024], bf16, tag="htbd")
            if l == 0:
                # bufs=2: first two allocations of each tag need zeroing;
                # later ones inherit the (never-overwritten) zero blocks
                nc.gpsimd.memset(hbd_n, 0.0)
                nc.vector.memset(htbd_n, 0.0)
            routing_layer(f"L{l}", 64, blogp[:, l * 512:(l + 1) * 512],
                          gp2, w1tp, hbd_c, htbd_c, hbd_n, htbd_n, None)
            hbd_c, htbd_c = hbd_n, htbd_n
        vout_sb = work.tile([64, 160], f32, tag="vo")
        routing_layer("F", 10, blog2p, g2p2, w2tp, hbd_c, htbd_c,
                      None, None, vout_sb)
        nc.sync.dma_start(out=vout_d[:, :], in_=vout_sb)

    nc.compile()
    return nc


def _prep_inputs(x, Wb, bb, W1, W2, b_basic, b_cls):
    """Host-side shard + relayout. Returns list of per-core input dicts."""
    import ml_dtypes
    f = np.float32
    bf = ml_dtypes.bfloat16

    wbp = np.ascontiguousarray(Wb.transpose(1, 2, 3, 0).reshape(64, 576), f)
    bbp = np.ascontiguousarray(np.tile(bb.reshape(1, 64), (2, 1))
                               .reshape(128, 1), f)

    def dup_blockdiag(mats):
        # mats: [o, 64, 64] -> [128, o*128] with diag(M_o, M_o)
        n = mats.shape[0]
        out = np.zeros((n, 128, 128), f)
        out[:, :64, :64] = mats
        out[:, 64:, 64:] = mats
        return np.ascontiguousarray(
            out.transpose(1, 0, 2).reshape(128, n * 128)).astype(bf)

    w1r = W1.reshape(64, 64, 64)                          # [o, d, c]
    g1 = np.einsum("odc,ode->oce", w1r, w1r)              # [o, c, e]
    gp2 = dup_blockdiag(g1)
    w1tp = dup_blockdiag(w1r.transpose(0, 2, 1))          # lhsT = [c, d]
    w2r = W2.reshape(10, 64, 64)
    g2 = np.einsum("odc,ode->oce", w2r, w2r)
    g2p2 = dup_blockdiag(g2)
    w2tp = dup_blockdiag(w2r.transpose(0, 2, 1))

    maps = []
    for core in range(NCORES):
        s = slice(core * B, (core + 1) * B)
        xs = x[s]                                         # [16,64,8,8]
        xpad = np.zeros((64, B, 10, 10), f)
        xpad[:, :, 1:9, 1:9] = xs.transpose(1, 0, 2, 3)
        xp = np.ascontiguousarray(xpad.reshape(64, 1600), f)
        # blogp[i + 64p, l*512 + q*64 + o] = b_basic[l, 2q+p, o, i]
        bs = b_basic[:, s]                                # [3,16,64,64]
        bq = bs.reshape(3, 8, 2, 64, 64)                  # [l,q,p,o,i]
        blogp = np.ascontiguousarray(
            bq.transpose(2, 4, 0, 1, 3).reshape(128, 1536)).astype(bf)
        cs = b_cls[s].reshape(8, 2, 10, 64)               # [q,p,o,i]
        blog2p = np.ascontiguousarray(
            cs.transpose(1, 3, 0, 2).reshape(128, 80)).astype(bf)
        maps.append(dict(xp=xp, wbp=wbp, bbp=bbp, gp2=gp2, w1tp=w1tp,
                         g2p2=g2p2, w2tp=w2tp, blogp=blogp, blog2p=blog2p))
    return maps


def kernel(x, Wb, bb, W1, b1, W2, b2, b_basic, b_cls):
    from concourse.bass_utils import run_bass_kernel_spmd

    if "nc" not in _PROG_CACHE:
        _PROG_CACHE["nc"] = _build_nc()
    nc = _PROG_CACHE["nc"]

    in_maps = _prep_inputs(np.asarray(x), np.asarray(Wb), np.asarray(bb),
                           np.asarray(W1), np.asarray(W2),
                           np.asarray(b_basic), np.asarray(b_cls))
    res = run_bass_kernel_spmd(nc, in_maps, list(range(NCORES)))
    out = np.empty((128, 10, 64), np.float32)
    for core in range(NCORES):
        vo = res.results[core]["vout"]                    # [64, 160]
        out[core * B:(core + 1) * B] = vo.reshape(64, B, 10).transpose(1, 2, 0)
    return out
